# revision 11
# baseline (speedup 1.0000x reference)
"""Trainium2 Bass kernel for a pre-norm transformer encoder layer with RoPE,
causal attention and SwiGLU FFN.

Sharding: data-parallel over batch (B=8 -> 8 NeuronCores, one batch element
per core).  Each core runs the full layer on its [S=1300, D=1080] slice.

Per-core dataflow (feature-major activations for matmuls):
  P1  LN1 on token-major x, PE-transpose -> x2T (f32r)      [actT]
  P2  V = x2 @ Wv  (token-major), staged to DRAM scratch    [vscr]
  P3  per head: Q/K proj (M=90) + RoPE (rotation matmul), scoresT = K.Q^T,
      E = exp(scoresT/sqrt(dk)) with causal zero-mask (affine_select),
      attnT = V^T.E with ones-matmul denominator, normalize via gpsimd
      partition_broadcast, stage attnT to DRAM scratch      [ascr]
  P4  out-proj accumulated over heads, PE-transpose back to token-major,
      r1 = x + attn_out + bo                                [r1]
  P5  LN2 on r1, transpose -> x2'T (overwrites actT)
  P6/P7 (two 128-aligned S-halves): H = silu(x2'@W1+b1)*(x2'@W3+b3) (bf16),
      ffn_out = H^T.W2 + b2, transpose, r1 += ; DMA r1 -> out

All matmuls run in float32r (full PE rate, ~1.6e-4 rel err) except the W2
matmul which uses bf16 (H and W2 are bf16 to save SBUF/HBM).
"""

import sys

sys.path.insert(0, "/opt/trn_rl_repo")

import math

import numpy as np

B, S, D, H, DK, FF = 8, 1300, 1080, 12, 90, 3240
EPS = 1e-5

N_ST = (S + 127) // 128                      # 11 token tiles
SW = [128] * (N_ST - 1) + [S - 128 * (N_ST - 1)]   # last = 20
N_KT = (D + 127) // 128                      # 9
KP = [128] * (N_KT - 1) + [D - 128 * (N_KT - 1)]   # last = 56
QCH = [(0, 512), (512, 512), (1024, 276)]    # q chunks (128-aligned starts)
MT = 120
N_MT = D // MT                               # 9
N_FT = (FF + 127) // 128                     # 26
FSZ = [128] * (N_FT - 1) + [FF - 128 * (N_FT - 1)]  # last = 40
N_VB = 3
VBW = D // N_VB                              # 360
FH = [(0, 384), (384, 384), (768, 384), (1152, 148)]  # ffn S chunks (128-aligned)

_CACHE = {}


def _build():
    from contextlib import ExitStack

    import concourse.bacc as bacc
    import concourse.mybir as mybir
    import concourse.tile as tile

    f32 = mybir.dt.float32
    f32r = mybir.dt.float32r
    bf16 = mybir.dt.bfloat16
    AF = mybir.ActivationFunctionType
    OP = mybir.AluOpType

    nc = bacc.Bacc("TRN2", target_bir_lowering=False, debug=False)

    def din(name, shape, dt=f32):
        return nc.dram_tensor(name, shape, dt, kind="ExternalInput").ap()

    def dout(name, shape, dt=f32):
        return nc.dram_tensor(name, shape, dt, kind="ExternalOutput").ap()

    x_d = din("x", (S, D))
    wq_d = din("wq", (D, D), f32r)
    wk_d = din("wk", (D, D), f32r)
    wv_d = din("wv", (D, D), f32r)
    wo_d = din("wo", (D, D), f32r)
    w1_d = din("w1", (D, FF), f32r)
    w3_d = din("w3", (D, FF), f32r)
    w2_d = din("w2", (FF, D), f32r)
    bq_d = din("bq", (D,))
    bk_d = din("bk", (D,))
    bv_d = din("bv", (D,))
    bo_d = din("bo", (D,))
    b1_d = din("b1", (FF,))
    b3_d = din("b3", (FF,))
    b2_d = din("b2", (D,))
    g1_d = din("g1", (D,))
    be1_d = din("be1", (D,))
    g2_d = din("g2", (D,))
    be2_d = din("be2", (D,))
    cost_d = din("cost", (DK, S))
    sint_d = din("sint", (DK, S))
    rl_d = din("rl", (DK, DK), f32r)
    ident_d = din("ident", (128, 128))

    out_d = dout("out", (S, D))
    vscr_d = dout("vscr", (S, D), f32r)
    ascr_d = dout("ascr", (H, DK, S), f32r)

    SCALE = 1.0 / math.sqrt(DK)

    with tile.TileContext(nc) as tc, ExitStack() as ctx:
        glob = ctx.enter_context(tc.tile_pool(name="glob", bufs=1))
        work = ctx.enter_context(tc.tile_pool(name="work", bufs=2))
        psA = ctx.enter_context(tc.tile_pool(name="psA", bufs=2, space="PSUM"))
        psB = ctx.enter_context(tc.tile_pool(name="psB", bufs=2, space="PSUM"))
        psC = ctx.enter_context(tc.tile_pool(name="psC", bufs=2, space="PSUM"))
        psT = ctx.enter_context(tc.tile_pool(name="psT", bufs=2, space="PSUM"))

        # ---------- persistent tensors ----------
        actT = glob.tile([128, N_KT, S], f32r, tag="actT")    # x2T / x2'T
        r1 = glob.tile([128, N_ST, D], f32, tag="r1")         # x -> x+attn -> +ffn

        # ---------- constants ----------
        ident = glob.tile([128, 128], f32, tag="ident")
        nc.sync.dma_start(ident, ident_d)
        rl_s = glob.tile([DK, DK], f32r, tag="rl")
        nc.sync.dma_start(rl_s, rl_d)
        cosT = glob.tile([DK, S], f32, tag="cosT")
        nc.sync.dma_start(cosT, cost_d)
        sinT = glob.tile([DK, S], f32, tag="sinT")
        nc.sync.dma_start(sinT, sint_d)
        ones_t = glob.tile([128, 1], f32r, tag="ones")
        nc.vector.memset(ones_t.bitcast(f32), 1.0)
        eps_t = glob.tile([128, 1], f32, tag="eps")
        nc.vector.memset(eps_t, EPS)

        def col_param(name, dram, n, psz):
            """[N]-vector -> [128, ntiles] sbuf; tile i holds psz[i] rows."""
            t = glob.tile([128, n], f32, tag=name, name=name)
            full = sum(1 for p in psz if p == 128)
            if full:
                nc.sync.dma_start(
                    t[:, :full],
                    dram[0 : 128 * full].rearrange("(o p) -> p o", p=128),
                )
            for i in range(full, n):
                o = sum(psz[:i])
                nc.sync.dma_start(t[: psz[i], i : i + 1], dram[o : o + psz[i], None])
            return t

        g1_s = col_param("g1", g1_d, N_KT, KP)
        be1_s = col_param("be1", be1_d, N_KT, KP)
        g2_s = col_param("g2", g2_d, N_KT, KP)
        be2_s = col_param("be2", be2_d, N_KT, KP)
        b1_s = col_param("b1", b1_d, N_FT, FSZ)
        b3_s = col_param("b3", b3_d, N_FT, FSZ)
        bo_s = glob.tile([MT, N_MT], f32, tag="bo")
        nc.sync.dma_start(bo_s, bo_d.rearrange("(o p) -> p o", p=MT))
        b2_s = glob.tile([MT, N_MT], f32, tag="b2")
        nc.sync.dma_start(b2_s, b2_d.rearrange("(o p) -> p o", p=MT))
        bq_s = glob.tile([DK, H], f32, tag="bq")
        nc.sync.dma_start(bq_s, bq_d.rearrange("(o p) -> p o", p=DK))
        bk_s = glob.tile([DK, H], f32, tag="bk")
        nc.sync.dma_start(bk_s, bk_d.rearrange("(o p) -> p o", p=DK))
        bv_row = glob.tile([1, D], f32, tag="bvrow")
        nc.sync.dma_start(bv_row, bv_d[None, :])
        bv_bc = glob.tile([128, D], f32, tag="bvbc")
        nc.gpsimd.partition_broadcast(bv_bc, bv_row)

        # ---------- helper: LN + transpose into actT ----------
        def layernorm_transpose(g_s, be_s):
            for st in range(N_ST):
                sw = SW[st]
                s0 = 128 * st
                xt = r1[:sw, st, :]
                ssum = work.tile([128, 1], f32, tag="ssum")
                nc.vector.reduce_sum(ssum[:sw], xt, axis=mybir.AxisListType.X)
                sqd = glob.tile([128, D], f32, tag="sqdump")
                ssq = work.tile([128, 1], f32, tag="ssq")
                nc.scalar.activation(sqd[:sw], xt, AF.Square, accum_out=ssq[:sw])
                mean = work.tile([128, 1], f32, tag="mean")
                nc.scalar.mul(mean[:sw], ssum[:sw], 1.0 / D)
                msq = work.tile([128, 1], f32, tag="msq")
                nc.vector.tensor_mul(msq[:sw], mean[:sw], mean[:sw])
                var = work.tile([128, 1], f32, tag="var")
                nc.vector.tensor_scalar_mul(var[:sw], ssq[:sw], 1.0 / D)
                nc.vector.tensor_sub(var[:sw], var[:sw], msq[:sw])
                std = work.tile([128, 1], f32, tag="std")
                nc.scalar.activation(std[:sw], var[:sw], AF.Sqrt, bias=eps_t[:sw])
                rstd = work.tile([128, 1], f32, tag="rstd")
                nc.vector.reciprocal(rstd[:sw], std[:sw])
                xn = work.tile([128, D], f32, tag="xn")
                nc.vector.tensor_scalar(
                    xn[:sw], xt, scalar1=mean[:sw], scalar2=rstd[:sw],
                    op0=OP.subtract, op1=OP.mult,
                )
                for kt in range(N_KT):
                    kp = KP[kt]
                    d0 = 128 * kt
                    pt = psT.tile([128, 128], f32, tag="pst")
                    nc.tensor.transpose(
                        pt[:kp, :sw], xn[:sw, d0 : d0 + kp], ident[:sw, :sw]
                    )
                    nc.scalar.activation(
                        actT[:kp, kt, s0 : s0 + sw],
                        pt[:kp, :sw],
                        AF.Identity,
                        bias=be_s[:kp, kt : kt + 1],
                        scale=g_s[:kp, kt : kt + 1],
                    )

        # ================= P1: LN1 =================
        for st in range(N_ST):
            nc.sync.dma_start(r1[: SW[st], st, :], x_d[128 * st : 128 * st + SW[st], :])
        layernorm_transpose(g1_s, be1_s)

        # ================= P2: V (token-major) -> vscr =================
        with tc.tile_pool(name="pP2", bufs=1) as pP2, \
             tc.tile_pool(name="pP2w", bufs=2) as pP2w:
            wv_t = pP2.tile([128, N_KT, D], f32r, tag="wv")
            nc.sync.dma_start(
                wv_t[:, : N_KT - 1, :],
                wv_d[0 : 128 * (N_KT - 1), :].rearrange("(o p) m -> p o m", p=128),
            )
            nc.sync.dma_start(wv_t[: KP[-1], N_KT - 1, :], wv_d[128 * (N_KT - 1) :, :])
            for st in range(N_ST):
                sw = SW[st]
                s0 = 128 * st
                for vb in range(N_VB):
                    c0 = VBW * vb
                    pv = psA.tile([128, VBW], f32, tag="pa")
                    for kt in range(N_KT):
                        kp = KP[kt]
                        nc.tensor.matmul(
                            pv[:sw],
                            actT[:kp, kt, s0 : s0 + sw],
                            wv_t[:kp, kt, c0 : c0 + VBW],
                            start=(kt == 0),
                            stop=(kt == N_KT - 1),
                        )
                    vsb = pP2w.tile([128, VBW], f32r, tag="vsb")
                    nc.vector.tensor_tensor(
                        vsb[:sw], pv[:sw], bv_bc[:sw, c0 : c0 + VBW], OP.add
                    )
                    nc.sync.dma_start(vscr_d[s0 : s0 + sw, c0 : c0 + VBW], vsb[:sw])

        # ================= P3: attention =================
        with tc.tile_pool(name="pP3", bufs=2) as pP3, \
             tc.tile_pool(name="pP3e", bufs=4) as pP3e:
            for h in range(H):
                c0 = DK * h
                wq_t = pP3.tile([128, N_KT, DK], f32r, tag="wq")
                nc.sync.dma_start(
                    wq_t[:, : N_KT - 1, :],
                    wq_d[0 : 128 * (N_KT - 1), c0 : c0 + DK].rearrange(
                        "(o p) m -> p o m", p=128
                    ),
                )
                nc.sync.dma_start(
                    wq_t[: KP[-1], N_KT - 1, :], wq_d[128 * (N_KT - 1) :, c0 : c0 + DK]
                )
                wk_t = pP3.tile([128, N_KT, DK], f32r, tag="wk")
                nc.sync.dma_start(
                    wk_t[:, : N_KT - 1, :],
                    wk_d[0 : 128 * (N_KT - 1), c0 : c0 + DK].rearrange(
                        "(o p) m -> p o m", p=128
                    ),
                )
                nc.sync.dma_start(
                    wk_t[: KP[-1], N_KT - 1, :], wk_d[128 * (N_KT - 1) :, c0 : c0 + DK]
                )

                qT = pP3.tile([DK, S], f32r, tag="qT")
                kT = pP3.tile([DK, S], f32r, tag="kT")
                for (w_t, b_s, outT) in ((wq_t, bq_s, qT), (wk_t, bk_s, kT)):
                    for (q0, qw) in QCH:
                        pq = psT.tile([DK, 512], f32, tag="pst")
                        for kt in range(N_KT):
                            kp = KP[kt]
                            nc.tensor.matmul(
                                pq[:, :qw],
                                w_t[:kp, kt, :],
                                actT[:kp, kt, q0 : q0 + qw],
                                start=(kt == 0),
                                stop=(kt == N_KT - 1),
                            )
                        raw = pP3.tile([DK, 512], f32r, tag="qraw")
                        nc.scalar.activation(
                            raw[:, :qw], pq[:, :qw], AF.Identity,
                            bias=b_s[:, h : h + 1],
                        )
                        prot = psT.tile([DK, 512], f32, tag="pst")
                        nc.tensor.matmul(
                            prot[:, :qw], rl_s, raw[:, :qw], start=True, stop=True
                        )
                        t1 = pP3.tile([DK, 512], f32, tag="ropet1")
                        nc.vector.tensor_tensor(
                            t1[:, :qw], raw[:, :qw].bitcast(f32),
                            cosT[:, q0 : q0 + qw], OP.mult,
                        )
                        t2 = pP3.tile([DK, 512], f32, tag="ropet2")
                        nc.vector.tensor_tensor(
                            t2[:, :qw], prot[:, :qw], sinT[:, q0 : q0 + qw], OP.mult
                        )
                        nc.vector.tensor_tensor(
                            outT[:, q0 : q0 + qw], t1[:, :qw], t2[:, :qw], OP.add
                        )

                vh = pP3.tile([128, N_ST, DK + 1], f32r, tag="vh")
                nc.sync.dma_start(
                    vh[:, : N_ST - 1, :DK],
                    vscr_d[0 : 128 * (N_ST - 1), c0 : c0 + DK].rearrange(
                        "(o p) m -> p o m", p=128
                    ),
                )
                nc.sync.dma_start(
                    vh[: SW[-1], N_ST - 1, :DK],
                    vscr_d[128 * (N_ST - 1) :, c0 : c0 + DK],
                )

                for (q0, qw) in QCH:
                    kmax = min(N_ST, (q0 + qw + 127) // 128)
                    pat = psB.tile([DK, 512], f32, tag="pb")
                    pden = psC.tile([1, 512], f32, tag="pc")
                    for i in range(kmax):
                        ksz = SW[i]
                        pe = psA.tile([128, 512], f32, tag="pa")
                        nc.tensor.matmul(
                            pe[:ksz, :qw],
                            kT[:, 128 * i : 128 * i + ksz],
                            qT[:, q0 : q0 + qw],
                            start=True,
                            stop=True,
                        )
                        et = pP3e.tile([128, 512], f32r, tag="et")
                        nc.scalar.activation(
                            et[:ksz, :qw], pe[:ksz, :qw], AF.Exp, scale=SCALE
                        )
                        if 128 * i + ksz - 1 > q0:
                            nc.gpsimd.affine_select(
                                out=et[:ksz, :qw],
                                in_=et[:ksz, :qw],
                                compare_op=OP.is_ge,
                                fill=0.0,
                                base=q0 - 128 * i,
                                pattern=[[1, qw]],
                                channel_multiplier=-1,
                            )
                        nc.tensor.matmul(
                            pat[:, :qw], vh[:ksz, i, :DK], et[:ksz, :qw],
                            start=(i == 0), stop=(i == kmax - 1),
                        )
                        nc.tensor.matmul(
                            pden[:, :qw], ones_t[:ksz], et[:ksz, :qw],
                            start=(i == 0), stop=(i == kmax - 1),
                        )
                    rec = pP3.tile([1, 512], f32, tag="rec")
                    nc.vector.reciprocal(rec[:, :qw], pden[:, :qw])
                    bc = pP3.tile([DK, 512], f32, tag="bc")
                    nc.gpsimd.partition_broadcast(bc[:, :qw], rec[:, :qw])
                    asb = pP3.tile([DK, 512], f32r, tag="asb")
                    nc.vector.tensor_tensor(
                        asb[:, :qw], pat[:, :qw], bc[:, :qw], OP.mult
                    )
                    nc.sync.dma_start(ascr_d[h, :, q0 : q0 + qw], asb[:, :qw])

        # ================= P4: out-proj + residual =================
        with tc.tile_pool(name="pP4", bufs=1) as pP4, \
             tc.tile_pool(name="pP4w", bufs=2) as pP4w:
            for (q0, qw) in QCH:
                arhs = [
                    pP4.tile([DK, 512], f32r, tag=f"ar{hh}", name=f"arhs{hh}")
                    for hh in range(H)
                ]
                for hh in range(H):
                    nc.sync.dma_start(arhs[hh][:, :qw], ascr_d[hh, :, q0 : q0 + qw])
                for mt in range(N_MT):
                    m0 = MT * mt
                    po = psA.tile([MT, 512], f32, tag="pa")
                    for hh in range(H):
                        wo_t = pP4w.tile([DK, MT], f32r, tag="wo")
                        nc.sync.dma_start(
                            wo_t, wo_d[DK * hh : DK * (hh + 1), m0 : m0 + MT]
                        )
                        nc.tensor.matmul(
                            po[:, :qw], wo_t, arhs[hh][:, :qw],
                            start=(hh == 0), stop=(hh == H - 1),
                        )
                    osb = pP4w.tile([MT, 512], f32, tag="osb")
                    nc.scalar.activation(
                        osb[:, :qw], po[:, :qw], AF.Identity,
                        bias=bo_s[:, mt : mt + 1],
                    )
                    for j in range((qw + 127) // 128):
                        st = (q0 + 128 * j) // 128
                        sw = min(128, qw - 128 * j)
                        ptr = psT.tile([128, MT], f32, tag="pst")
                        nc.tensor.transpose(
                            ptr[:sw, :], osb[:, 128 * j : 128 * j + sw],
                            ident[:MT, :MT],
                        )
                        nc.vector.tensor_tensor(
                            r1[:sw, st, m0 : m0 + MT], r1[:sw, st, m0 : m0 + MT],
                            ptr[:sw, :], OP.add,
                        )

        # ================= P5: LN2 =================
        layernorm_transpose(g2_s, be2_s)

        # ================= P6/P7: FFN in S chunks (all f32r) =================
        with tc.tile_pool(name="pF", bufs=1) as pF, \
             tc.tile_pool(name="pFw", bufs=2) as pFw, \
             tc.tile_pool(name="pFw2", bufs=1) as pFw2:
            for (hq0, hqw) in FH:
                Ht = pF.tile([128, N_FT, 384], f32r, tag="Ht", name="Ht")
                for ft in range(N_FT):
                    fsz = FSZ[ft]
                    f0 = 128 * ft
                    w1_t = pFw.tile([128, N_KT, 128], f32r, tag="w1")
                    nc.sync.dma_start(
                        w1_t[:, : N_KT - 1, :fsz],
                        w1_d[0 : 128 * (N_KT - 1), f0 : f0 + fsz].rearrange(
                            "(o p) m -> p o m", p=128
                        ),
                    )
                    nc.sync.dma_start(
                        w1_t[: KP[-1], N_KT - 1, :fsz],
                        w1_d[128 * (N_KT - 1) :, f0 : f0 + fsz],
                    )
                    w3_t = pFw.tile([128, N_KT, 128], f32r, tag="w3")
                    nc.sync.dma_start(
                        w3_t[:, : N_KT - 1, :fsz],
                        w3_d[0 : 128 * (N_KT - 1), f0 : f0 + fsz].rearrange(
                            "(o p) m -> p o m", p=128
                        ),
                    )
                    nc.sync.dma_start(
                        w3_t[: KP[-1], N_KT - 1, :fsz],
                        w3_d[128 * (N_KT - 1) :, f0 : f0 + fsz],
                    )
                    p1_ = psA.tile([128, 512], f32, tag="pa")
                    p3_ = psB.tile([128, 512], f32, tag="pb")
                    for kt in range(N_KT):
                        kp = KP[kt]
                        nc.tensor.matmul(
                            p1_[:fsz, :hqw], w1_t[:kp, kt, :fsz],
                            actT[:kp, kt, hq0 : hq0 + hqw],
                            start=(kt == 0), stop=(kt == N_KT - 1),
                        )
                        nc.tensor.matmul(
                            p3_[:fsz, :hqw], w3_t[:kp, kt, :fsz],
                            actT[:kp, kt, hq0 : hq0 + hqw],
                            start=(kt == 0), stop=(kt == N_KT - 1),
                        )
                    h1s = pFw.tile([128, 512], f32, tag="h1s")
                    nc.scalar.activation(
                        h1s[:fsz, :hqw], p1_[:fsz, :hqw], AF.Silu,
                        bias=b1_s[:fsz, ft : ft + 1],
                    )
                    h3b = pFw.tile([128, 512], f32, tag="h3b")
                    nc.scalar.activation(
                        h3b[:fsz, :hqw], p3_[:fsz, :hqw], AF.Identity,
                        bias=b3_s[:fsz, ft : ft + 1],
                    )
                    nc.vector.tensor_tensor(
                        Ht[:fsz, ft, :hqw], h1s[:fsz, :hqw], h3b[:fsz, :hqw],
                        OP.mult,
                    )
                for mt in range(N_MT):
                    m0 = MT * mt
                    w2_t = pFw2.tile([128, N_FT, MT], f32r, tag="w2")
                    nc.sync.dma_start(
                        w2_t[:, : N_FT - 1, :],
                        w2_d[0 : 128 * (N_FT - 1), m0 : m0 + MT].rearrange(
                            "(o p) m -> p o m", p=128
                        ),
                    )
                    nc.sync.dma_start(
                        w2_t[: FSZ[-1], N_FT - 1, :],
                        w2_d[128 * (N_FT - 1) :, m0 : m0 + MT],
                    )
                    pf = psA.tile([MT, 512], f32, tag="pa")
                    for ft in range(N_FT):
                        fsz = FSZ[ft]
                        nc.tensor.matmul(
                            pf[:, :hqw], w2_t[:fsz, ft, :],
                            Ht[:fsz, ft, :hqw],
                            start=(ft == 0), stop=(ft == N_FT - 1),
                        )
                    fsb = pFw.tile([MT, 512], f32, tag="fsb")
                    nc.scalar.activation(
                        fsb[:, :hqw], pf[:, :hqw], AF.Identity,
                        bias=b2_s[:, mt : mt + 1],
                    )
                    for j in range((hqw + 127) // 128):
                        st = (hq0 + 128 * j) // 128
                        sw = min(128, hqw - 128 * j)
                        ptr = psT.tile([128, MT], f32, tag="pst")
                        nc.tensor.transpose(
                            ptr[:sw, :], fsb[:, 128 * j : 128 * j + sw],
                            ident[:MT, :MT],
                        )
                        nc.vector.tensor_tensor(
                            r1[:sw, st, m0 : m0 + MT], r1[:sw, st, m0 : m0 + MT],
                            ptr[:sw, :], OP.add,
                        )
        for st in range(N_ST):
            nc.sync.dma_start(
                out_d[128 * st : 128 * st + SW[st], :], r1[: SW[st], st, :]
            )

    nc.compile()
    return nc


def _host_inputs(inputs):
    """Shared (per-core-identical) input map pieces, from full inputs."""
    import ml_dtypes

    cos = np.ascontiguousarray(np.asarray(inputs["rope_cos"], np.float32).T)
    sin = np.ascontiguousarray(np.asarray(inputs["rope_sin"], np.float32).T)
    rl = np.zeros((DK, DK), np.float32)
    hdk = DK // 2
    rl[np.arange(hdk) + hdk, np.arange(hdk)] = -1.0
    rl[np.arange(hdk), np.arange(hdk) + hdk] = 1.0
    ident = np.eye(128, dtype=np.float32)
    f = lambda k: np.ascontiguousarray(np.asarray(inputs[k], np.float32))
    return {
        "wq": f("Wq"), "wk": f("Wk"), "wv": f("Wv"), "wo": f("Wo"),
        "w1": f("W1"), "w3": f("W3"),
        "w2": f("W2"),
        "bq": f("bq"), "bk": f("bk"), "bv": f("bv"), "bo": f("bo"),
        "b1": f("b1"), "b3": f("b3"), "b2": f("b2"),
        "g1": f("ln1_g"), "be1": f("ln1_b"), "g2": f("ln2_g"), "be2": f("ln2_b"),
        "cost": cos, "sint": sin, "rl": rl, "ident": ident,
    }


def kernel(**inputs):
    from concourse.bass_utils import run_bass_kernel_spmd

    if "nc" not in _CACHE:
        _CACHE["nc"] = _build()
    nc = _CACHE["nc"]

    shared = _host_inputs(inputs)
    x = np.asarray(inputs["x"], np.float32)
    in_maps = [dict(shared, x=np.ascontiguousarray(x[b])) for b in range(B)]
    res = run_bass_kernel_spmd(nc, in_maps, list(range(B))).results
    out = np.stack([res[b]["out"] for b in range(B)], axis=0)
    return out.astype(np.float32)


# revision 24
# speedup vs baseline: 960.1671x; 960.1671x over previous
"""Trainium2 Bass kernel for a pre-norm transformer encoder layer with RoPE,
causal attention and SwiGLU FFN.

Sharding: data-parallel over batch (B=8 -> 8 NeuronCores, one batch element
per core).  Each core runs the full layer on its [S=1300, D=1080] slice.

Per-core dataflow (feature-major activations for matmuls):
  P1  LN1 on token-major x, PE-transpose -> x2T (f32r)      [actT]
  P2  V = x2 @ Wv  (token-major), staged to DRAM scratch    [vscr]
  P3  per head: Q/K proj (M=90) + RoPE (rotation matmul), scoresT = K.Q^T,
      E = exp(scoresT/sqrt(dk)) with causal zero-mask (affine_select),
      attnT = V^T.E with ones-matmul denominator, normalize via gpsimd
      partition_broadcast, stage attnT to DRAM scratch      [ascr]
  P4  out-proj accumulated over heads, PE-transpose back to token-major,
      r1 = x + attn_out + bo                                [r1]
  P5  LN2 on r1, transpose -> x2'T (overwrites actT)
  P6/P7 (two 128-aligned S-chunks): H = silu(x2'@W1+b1)*(x2'@W3+b3),
      ffn_out = H^T.W2 + b2, transpose, accumulate-DMA into out

All matmuls run in float32r: full PE rate (1 cyc/row for N>=256) at ~1.6e-4
relative error.  Weights are passed from the host in pre-rearranged, padded
layouts so every weight DMA is a single fully-contiguous transfer.
"""

import sys

sys.path.insert(0, "/opt/trn_rl_repo")

import math

import numpy as np

B, S, D, H, DK, FF = 8, 1300, 1080, 12, 90, 3240
EPS = 1e-5

N_ST = (S + 127) // 128                      # 11 token tiles
SW = [128] * (N_ST - 1) + [S - 128 * (N_ST - 1)]   # last = 20
N_KT = (D + 127) // 128                      # 9
KP = [128] * (N_KT - 1) + [D - 128 * (N_KT - 1)]   # last = 56
QCH = [(0, 512), (512, 512), (1024, 276)]    # q chunks (128-aligned starts)
MT = 120
N_MT = D // MT                               # 9
N_FT = (FF + 127) // 128                     # 26
FSZ = [128] * (N_FT - 1) + [FF - 128 * (N_FT - 1)]  # last = 40
N_VB = 3
VBW = D // N_VB                              # 360
FH = [(0, 640), (640, 660)]                  # ffn S chunks (128-aligned)

_CACHE = {}


def _build():
    from contextlib import ExitStack

    import concourse.bacc as bacc
    import concourse.mybir as mybir
    import concourse.tile as tile

    f32 = mybir.dt.float32
    f32r = mybir.dt.float32r
    bf16 = mybir.dt.bfloat16
    AF = mybir.ActivationFunctionType
    OP = mybir.AluOpType

    nc = bacc.Bacc("TRN2", target_bir_lowering=False, debug=False)

    def din(name, shape, dt=f32):
        return nc.dram_tensor(name, shape, dt, kind="ExternalInput").ap()

    def dout(name, shape, dt=f32):
        return nc.dram_tensor(name, shape, dt, kind="ExternalOutput").ap()

    x_d = din("x", (S, D))
    wq_d = din("wqr", (H, 128, N_KT, DK), f32r)
    wk_d = din("wkr", (H, 128, N_KT, DK), f32r)
    wv_d = din("wvr", (128, N_KT, D), f32r)
    wo_d = din("wor", (H, N_MT, DK, MT), f32r)
    w1_d = din("w1r", (N_FT, 128, N_KT, 128), f32r)
    w3_d = din("w3r", (N_FT, 128, N_KT, 128), f32r)
    w2_d = din("w2r", (N_MT, 128, N_FT, MT), f32r)
    bq_d = din("bq", (D,))
    bk_d = din("bk", (D,))
    bv_d = din("bv", (D,))
    bo_d = din("bo", (D,))
    b1_d = din("b1", (FF,))
    b3_d = din("b3", (FF,))
    b2_d = din("b2", (D,))
    g1_d = din("g1", (D,))
    be1_d = din("be1", (D,))
    g2_d = din("g2", (D,))
    be2_d = din("be2", (D,))
    cost_d = din("cost", (DK, S))
    sint_d = din("sint", (DK, S))
    rl_d = din("rl", (DK, DK), f32r)
    ident_d = din("ident", (128, 128))
    cmask_d = din("cmask", (4, 128, 512))
    bvb_d = din("bvb", (128, D))
    onesrow_d = din("onesrow", (1, 128), f32r)

    out_d = dout("out", (S, D))
    vscr_d = dout("vscr", (H, 1408, DK), f32r)
    ascr_d = dout("ascr", (H, DK, S), f32r)

    SCALE = 1.0 / math.sqrt(DK)

    with tile.TileContext(nc) as tc, ExitStack() as ctx:
        glob = ctx.enter_context(tc.tile_pool(name="glob", bufs=1))
        work = ctx.enter_context(tc.tile_pool(name="work", bufs=2))
        psA = ctx.enter_context(tc.tile_pool(name="psA", bufs=2, space="PSUM"))
        psB = ctx.enter_context(tc.tile_pool(name="psB", bufs=2, space="PSUM"))
        psC = ctx.enter_context(tc.tile_pool(name="psC", bufs=2, space="PSUM"))
        psT = ctx.enter_context(tc.tile_pool(name="psT", bufs=2, space="PSUM"))

        # ---------- persistent tensors ----------
        actT = glob.tile([128, N_KT, S], f32r, tag="actT")    # x2T / x2'T
        pR1cm = tc.tile_pool(name="pR1", bufs=1)
        pR1 = pR1cm.__enter__()
        r1 = pR1.tile([128, N_ST, D], f32, tag="r1", name="r1")

        # ---------- constants ----------
        ident = glob.tile([128, 128], f32, tag="ident")
        nc.sync.dma_start(ident, ident_d)
        rl_s = glob.tile([DK, DK], f32r, tag="rl")
        nc.sync.dma_start(rl_s, rl_d)
        cosT = glob.tile([DK, S], f32, tag="cosT")
        nc.sync.dma_start(cosT, cost_d)
        sinT = glob.tile([DK, S], f32, tag="sinT")
        nc.sync.dma_start(sinT, sint_d)
        ones_t = glob.tile([128, 1], f32r, tag="ones")
        nc.vector.memset(ones_t.bitcast(f32), 1.0)
        eps_t = glob.tile([128, 1], f32, tag="eps")
        nc.vector.memset(eps_t, EPS)

        def col_param(name, dram, n, psz):
            """[N]-vector -> [128, ntiles] sbuf; tile i holds psz[i] rows."""
            t = glob.tile([128, n], f32, tag=name, name=name)
            full = sum(1 for p in psz if p == 128)
            if full:
                nc.sync.dma_start(
                    t[:, :full],
                    dram[0 : 128 * full].rearrange("(o p) -> p o", p=128),
                )
            for i in range(full, n):
                o = sum(psz[:i])
                nc.sync.dma_start(t[: psz[i], i : i + 1], dram[o : o + psz[i], None])
            return t

        g1_s = col_param("g1", g1_d, N_KT, KP)
        be1_s = col_param("be1", be1_d, N_KT, KP)
        g2_s = col_param("g2", g2_d, N_KT, KP)
        be2_s = col_param("be2", be2_d, N_KT, KP)
        b1_s = col_param("b1", b1_d, N_FT, FSZ)
        b3_s = col_param("b3", b3_d, N_FT, FSZ)
        bo_s = glob.tile([MT, N_MT], f32, tag="bo")
        nc.sync.dma_start(bo_s, bo_d.rearrange("(o p) -> p o", p=MT))
        b2_s = glob.tile([MT, N_MT], f32, tag="b2")
        nc.sync.dma_start(b2_s, b2_d.rearrange("(o p) -> p o", p=MT))
        bq_s = glob.tile([DK, H], f32, tag="bq")
        nc.sync.dma_start(bq_s, bq_d.rearrange("(o p) -> p o", p=DK))
        bk_s = glob.tile([DK, H], f32, tag="bk")
        nc.sync.dma_start(bk_s, bk_d.rearrange("(o p) -> p o", p=DK))
        bv_bc = glob.tile([128, D], f32, tag="bvbc")
        nc.sync.dma_start(bv_bc, bvb_d)
        cm_s = glob.tile([128, 4, 512], f32, tag="cmask")
        nc.sync.dma_start(cm_s, cmask_d.rearrange("t p f -> p t f"))
        ones_row = glob.tile([1, 128], f32r, tag="onesrow")
        nc.sync.dma_start(ones_row, onesrow_d)

        # ---------- helper: LN + transpose into actT ----------
        def layernorm_transpose(g_s, be_s):
            for st in range(N_ST):
                sw = SW[st]
                s0 = 128 * st
                xt = r1[:sw, st, :]
                ssum = work.tile([128, 1], f32, tag="ssum")
                nc.vector.reduce_sum(ssum[:sw], xt, axis=mybir.AxisListType.X)
                sqd = glob.tile([128, D], f32, tag="sqdump")
                ssq = work.tile([128, 1], f32, tag="ssq")
                nc.scalar.activation(sqd[:sw], xt, AF.Square, accum_out=ssq[:sw])
                mean = work.tile([128, 1], f32, tag="mean")
                nc.scalar.mul(mean[:sw], ssum[:sw], 1.0 / D)
                msq = work.tile([128, 1], f32, tag="msq")
                nc.vector.tensor_mul(msq[:sw], mean[:sw], mean[:sw])
                var = work.tile([128, 1], f32, tag="var")
                nc.vector.tensor_scalar_mul(var[:sw], ssq[:sw], 1.0 / D)
                nc.vector.tensor_sub(var[:sw], var[:sw], msq[:sw])
                std = work.tile([128, 1], f32, tag="std")
                nc.scalar.activation(std[:sw], var[:sw], AF.Sqrt, bias=eps_t[:sw])
                rstd = work.tile([128, 1], f32, tag="rstd")
                nc.vector.reciprocal(rstd[:sw], std[:sw])
                xn = work.tile([128, D], f32, tag="xn")
                nc.vector.tensor_scalar(
                    xn[:sw], xt, scalar1=mean[:sw], scalar2=rstd[:sw],
                    op0=OP.subtract, op1=OP.mult,
                )
                for kt in range(N_KT):
                    kp = KP[kt]
                    d0 = 128 * kt
                    pt = psT.tile([128, 128], f32, tag="pst")
                    nc.tensor.transpose(
                        pt[:kp, :sw], xn[:sw, d0 : d0 + kp], ident[:sw, :sw]
                    )
                    nc.scalar.activation(
                        actT[:kp, kt, s0 : s0 + sw],
                        pt[:kp, :sw],
                        AF.Identity,
                        bias=be_s[:kp, kt : kt + 1],
                        scale=g_s[:kp, kt : kt + 1],
                    )

        # ================= P1: LN1 =================
        for st in range(N_ST):
            nc.sync.dma_start(r1[: SW[st], st, :], x_d[128 * st : 128 * st + SW[st], :])
        layernorm_transpose(g1_s, be1_s)

        # ================= P2: V (token-major) -> vscr =================
        with tc.tile_pool(name="pP2", bufs=1) as pP2, \
             tc.tile_pool(name="pP2w", bufs=2) as pP2w:
            wv_t = pP2.tile([128, N_KT, D], f32r, tag="wv")
            nc.sync.dma_start(wv_t, wv_d)
            for st in range(N_ST):
                sw = SW[st]
                s0 = 128 * st
                for vb in range(N_VB):
                    c0 = VBW * vb
                    pv = psA.tile([128, VBW], f32, tag="pa")
                    for kt in range(N_KT):
                        kp = KP[kt]
                        nc.tensor.matmul(
                            pv[:sw],
                            actT[:kp, kt, s0 : s0 + sw],
                            wv_t[:kp, kt, c0 : c0 + VBW],
                            start=(kt == 0),
                            stop=(kt == N_KT - 1),
                        )
                    vsb = pP2w.tile([128, VBW], f32r, tag="vsb")
                    nc.vector.tensor_tensor(
                        vsb[:sw], pv[:sw], bv_bc[:sw, c0 : c0 + VBW], OP.add
                    )
                    nc.sync.dma_start(
                        vscr_d[4 * vb : 4 * vb + 4, s0 : s0 + sw, :].rearrange(
                            "h s d -> s h d"
                        ),
                        vsb[:sw],
                    )

        # ================= P3: attention =================
        with tc.tile_pool(name="pP3", bufs=2) as pP3, \
             tc.tile_pool(name="pP3e", bufs=6) as pP3e:
            for h in range(H):
                c0 = DK * h
                wq_t = pP3.tile([128, N_KT, DK], f32r, tag="wq")
                nc.sync.dma_start(wq_t, wq_d[h])
                wk_t = pP3.tile([128, N_KT, DK], f32r, tag="wk")
                nc.sync.dma_start(wk_t, wk_d[h])

                qT = pP3.tile([DK, S], f32r, tag="qT")
                kT = pP3.tile([DK, S], f32r, tag="kT")
                for (w_t, b_s, outT) in ((wq_t, bq_s, qT), (wk_t, bk_s, kT)):
                    for (q0, qw) in QCH:
                        pq = psT.tile([DK, 512], f32, tag="pst")
                        for kt in range(N_KT):
                            kp = KP[kt]
                            nc.tensor.matmul(
                                pq[:, :qw],
                                w_t[:kp, kt, :],
                                actT[:kp, kt, q0 : q0 + qw],
                                start=(kt == 0),
                                stop=(kt == N_KT - 1),
                            )
                        raw = pP3.tile([DK, 512], f32r, tag="qraw")
                        nc.scalar.activation(
                            raw[:, :qw], pq[:, :qw], AF.Identity,
                            bias=b_s[:, h : h + 1],
                        )
                        prot = psT.tile([DK, 512], f32, tag="pst")
                        nc.tensor.matmul(
                            prot[:, :qw], rl_s, raw[:, :qw], start=True, stop=True
                        )
                        t1 = pP3.tile([DK, 512], f32, tag="ropet1")
                        nc.vector.tensor_tensor(
                            t1[:, :qw], raw[:, :qw].bitcast(f32),
                            cosT[:, q0 : q0 + qw], OP.mult,
                        )
                        t2 = pP3.tile([DK, 512], f32, tag="ropet2")
                        nc.vector.tensor_tensor(
                            t2[:, :qw], prot[:, :qw], sinT[:, q0 : q0 + qw], OP.mult
                        )
                        nc.vector.tensor_tensor(
                            outT[:, q0 : q0 + qw], t1[:, :qw], t2[:, :qw], OP.add
                        )

                vh = pP3.tile([128, N_ST, DK + 1], f32r, tag="vh")
                nc.sync.dma_start(
                    vh[:, :, :DK],
                    vscr_d[h].rearrange("(o p) d -> p o d", p=128),
                )

                for (q0, qw) in QCH:
                    kmax = min(N_ST, (q0 + qw + 127) // 128)
                    pat = psB.tile([DK, 512], f32, tag="pb")
                    pden = psC.tile([1, 512], f32, tag="pc")
                    for i in range(kmax):
                        ksz = SW[i]
                        pe = psA.tile([128, 512], f32, tag="pa")
                        nc.tensor.matmul(
                            pe[:ksz, :qw],
                            kT[:, 128 * i : 128 * i + ksz],
                            qT[:, q0 : q0 + qw],
                            start=True,
                            stop=True,
                        )
                        et = pP3e.tile([128, 512], f32r, tag="et")
                        nc.scalar.activation(
                            et[:ksz, :qw], pe[:ksz, :qw], AF.Exp, scale=SCALE
                        )
                        if 128 * i + ksz - 1 > q0:
                            t_ = i - q0 // 128
                            nc.vector.tensor_tensor(
                                et[:ksz, :qw], et[:ksz, :qw],
                                cm_s[:ksz, t_, :qw], OP.mult,
                            )
                        nc.tensor.matmul(
                            pat[:, :qw], vh[:ksz, i, :DK], et[:ksz, :qw],
                            start=(i == 0), stop=(i == kmax - 1),
                        )
                        nc.tensor.matmul(
                            pden[:, :qw], ones_t[:ksz], et[:ksz, :qw],
                            start=(i == 0), stop=(i == kmax - 1),
                        )
                    rec = pP3.tile([1, 512], f32r, tag="rec")
                    with nc.allow_low_precision(reason="f32r denom bcast"):
                        nc.vector.reciprocal(rec[:, :qw], pden[:, :qw])
                    bcp = psC.tile([DK, 512], f32, tag="pc")
                    nc.tensor.matmul(
                        bcp[:, :qw], ones_row[:1, :DK], rec[:, :qw],
                        start=True, stop=True,
                    )
                    bc = pP3.tile([DK, 512], f32, tag="bc")
                    nc.vector.tensor_copy(bc[:, :qw], bcp[:, :qw])
                    asb = pP3.tile([DK, 512], f32r, tag="asb")
                    nc.vector.tensor_tensor(
                        asb[:, :qw], pat[:, :qw], bc[:, :qw], OP.mult
                    )
                    nc.sync.dma_start(ascr_d[h, :, q0 : q0 + qw], asb[:, :qw])

        # ================= P4: out-proj + residual =================
        ACH = [(0, 640), (640, 660)]
        with tc.tile_pool(name="pP4", bufs=1) as pP4, \
             tc.tile_pool(name="pP4w", bufs=1) as pP4w, \
             tc.tile_pool(name="pP4o", bufs=2) as pP4o:
            for (q0, qw) in ACH:
                half = qw // 2
                sub = [(0, half), (half, qw - half)]
                arhs = [
                    pP4.tile([DK, 660], f32r, tag=f"ar{hh}", name=f"arhs{hh}")
                    for hh in range(H)
                ]
                for hh in range(H):
                    nc.sync.dma_start(arhs[hh][:, :qw], ascr_d[hh, :, q0 : q0 + qw])
                for mt in range(N_MT):
                    m0 = MT * mt
                    osb = pP4o.tile([MT, 660], f32, tag="osb")
                    wo_ts = []
                    for hh in range(H):
                        wo_t = pP4w.tile(
                            [DK, MT], f32r, tag=f"wo{hh}", name=f"wo{hh}"
                        )
                        nc.sync.dma_start(wo_t, wo_d[hh, mt])
                        wo_ts.append(wo_t)
                    for (so, sw_) in sub:
                        po = psA.tile([MT, 512], f32, tag="pa")
                        for hh in range(H):
                            nc.tensor.matmul(
                                po[:, :sw_], wo_ts[hh], arhs[hh][:, so : so + sw_],
                                start=(hh == 0), stop=(hh == H - 1),
                            )
                        nc.scalar.activation(
                            osb[:, so : so + sw_], po[:, :sw_], AF.Identity,
                            bias=bo_s[:, mt : mt + 1],
                        )
                    for j in range((qw + 127) // 128):
                        st = (q0 + 128 * j) // 128
                        sw = min(128, qw - 128 * j)
                        ptr = psT.tile([128, MT], f32, tag="pst")
                        nc.tensor.transpose(
                            ptr[:sw, :], osb[:, 128 * j : 128 * j + sw],
                            ident[:MT, :MT],
                        )
                        nc.vector.tensor_tensor(
                            r1[:sw, st, m0 : m0 + MT], r1[:sw, st, m0 : m0 + MT],
                            ptr[:sw, :], OP.add,
                        )

        # ================= P5: LN2 =================
        layernorm_transpose(g2_s, be2_s)

        # r1 is complete (LN2 consumed it): stage base of output to DRAM so
        # the r1 slab frees up for the FFN hidden tensor.
        for st in range(N_ST):
            nc.sync.dma_start(
                out_d[128 * st : 128 * st + SW[st], :], r1[: SW[st], st, :]
            )

        pR1cm.__exit__(None, None, None)

        # ================= P6/P7: FFN in two S chunks (all f32r) =================
        with tc.tile_pool(name="pF", bufs=1) as pF, \
             tc.tile_pool(name="pFw", bufs=2) as pFw, \
             tc.tile_pool(name="pFw2", bufs=1) as pFw2:
            for (hq0, hqw) in FH:
                Ht = pF.tile([128, N_FT, 660], f32r, tag="Ht", name="Ht")
                half = hqw // 2
                sub = [(0, half), (half, hqw - half)]
                for ft in range(N_FT):
                    fsz = FSZ[ft]
                    w1_t = pFw.tile([128, N_KT, 128], f32r, tag="w1")
                    nc.sync.dma_start(w1_t, w1_d[ft])
                    w3_t = pFw.tile([128, N_KT, 128], f32r, tag="w3")
                    nc.sync.dma_start(w3_t, w3_d[ft])
                    for (so, sw_) in sub:
                        g0 = hq0 + so
                        p1_ = psA.tile([128, 512], f32, tag="pa")
                        p3_ = psB.tile([128, 512], f32, tag="pb")
                        for kt in range(N_KT):
                            kp = KP[kt]
                            nc.tensor.matmul(
                                p1_[:fsz, :sw_], w1_t[:kp, kt, :fsz],
                                actT[:kp, kt, g0 : g0 + sw_],
                                start=(kt == 0), stop=(kt == N_KT - 1),
                            )
                            nc.tensor.matmul(
                                p3_[:fsz, :sw_], w3_t[:kp, kt, :fsz],
                                actT[:kp, kt, g0 : g0 + sw_],
                                start=(kt == 0), stop=(kt == N_KT - 1),
                            )
                        h1s = pFw.tile([128, 512], f32, tag="h1s")
                        nc.scalar.activation(
                            h1s[:fsz, :sw_], p1_[:fsz, :sw_], AF.Silu,
                            bias=b1_s[:fsz, ft : ft + 1],
                        )
                        h3b = pFw.tile([128, 512], f32, tag="h3b")
                        nc.scalar.activation(
                            h3b[:fsz, :sw_], p3_[:fsz, :sw_], AF.Identity,
                            bias=b3_s[:fsz, ft : ft + 1],
                        )
                        nc.vector.tensor_tensor(
                            Ht[:fsz, ft, so : so + sw_], h1s[:fsz, :sw_],
                            h3b[:fsz, :sw_], OP.mult,
                        )
                for mt in range(N_MT):
                    m0 = MT * mt
                    w2_t = pFw2.tile([128, N_FT, MT], f32r, tag="w2")
                    nc.sync.dma_start(w2_t, w2_d[mt])
                    fsb = pFw.tile([MT, 660], f32, tag="fsb")
                    for (so, sw_) in sub:
                        pf = psA.tile([MT, 512], f32, tag="pa")
                        for ft in range(N_FT):
                            fsz = FSZ[ft]
                            nc.tensor.matmul(
                                pf[:, :sw_], w2_t[:fsz, ft, :],
                                Ht[:fsz, ft, so : so + sw_],
                                start=(ft == 0), stop=(ft == N_FT - 1),
                            )
                        nc.scalar.activation(
                            fsb[:, so : so + sw_], pf[:, :sw_], AF.Identity,
                            bias=b2_s[:, mt : mt + 1],
                        )
                    for j in range((hqw + 127) // 128):
                        st = (hq0 + 128 * j) // 128
                        sw = min(128, hqw - 128 * j)
                        ptr = psT.tile([128, MT], f32, tag="pst")
                        nc.tensor.transpose(
                            ptr[:sw, :], fsb[:, 128 * j : 128 * j + sw],
                            ident[:MT, :MT],
                        )
                        stage = pFw.tile([128, MT], f32, tag="stage")
                        nc.vector.tensor_copy(stage[:sw], ptr[:sw])
                        nc.gpsimd.dma_start(
                            out_d[hq0 + 128 * j : hq0 + 128 * j + sw, m0 : m0 + MT],
                            stage[:sw],
                            accum_op=OP.add,
                        )

    nc.compile()
    return nc


def _host_inputs(inputs):
    """Shared (per-core-identical) input map pieces, from full inputs."""
    cos = np.ascontiguousarray(np.asarray(inputs["rope_cos"], np.float32).T)
    sin = np.ascontiguousarray(np.asarray(inputs["rope_sin"], np.float32).T)
    rl = np.zeros((DK, DK), np.float32)
    hdk = DK // 2
    rl[np.arange(hdk) + hdk, np.arange(hdk)] = -1.0
    rl[np.arange(hdk), np.arange(hdk) + hdk] = 1.0
    ident = np.eye(128, dtype=np.float32)
    f = lambda k: np.ascontiguousarray(np.asarray(inputs[k], np.float32))

    def pad_rows(w, rows):
        out = np.zeros((rows, w.shape[1]), np.float32)
        out[: w.shape[0]] = w
        return out

    Wq = f("Wq"); Wk = f("Wk"); Wv = f("Wv"); Wo = f("Wo")
    W1 = f("W1"); W3 = f("W3"); W2 = f("W2")
    KR = N_KT * 128
    # [H, 128, N_KT, DK]: (h, p, o, d) = Wq[o*128+p, h*90+d]
    wqr = np.ascontiguousarray(
        pad_rows(Wq, KR).reshape(N_KT, 128, H, DK).transpose(2, 1, 0, 3))
    wkr = np.ascontiguousarray(
        pad_rows(Wk, KR).reshape(N_KT, 128, H, DK).transpose(2, 1, 0, 3))
    # [128, N_KT, D]
    wvr = np.ascontiguousarray(pad_rows(Wv, KR).reshape(N_KT, 128, D).transpose(1, 0, 2))
    # [H, N_MT, DK, MT]
    wor = np.ascontiguousarray(Wo.reshape(H, DK, N_MT, MT).transpose(0, 2, 1, 3))
    # [N_FT, 128, N_KT, 128]: (ft, p, o, m) = W1[o*128+p, ft*128+m]
    FR = N_FT * 128
    w1p = np.zeros((KR, FR), np.float32); w1p[:D, :FF] = W1
    w3p = np.zeros((KR, FR), np.float32); w3p[:D, :FF] = W3
    w1r = np.ascontiguousarray(
        w1p.reshape(N_KT, 128, N_FT, 128).transpose(2, 1, 0, 3))
    w3r = np.ascontiguousarray(
        w3p.reshape(N_KT, 128, N_FT, 128).transpose(2, 1, 0, 3))
    # [N_MT, 128, N_FT, MT]: (mt, p, o, m) = W2[o*128+p, mt*120+m]
    w2p = np.zeros((FR, D), np.float32); w2p[:FF] = W2
    w2r = np.ascontiguousarray(
        w2p.reshape(N_FT, 128, N_MT, MT).transpose(2, 1, 0, 3))
    cmask = np.zeros((4, 128, 512), np.float32)
    for t in range(4):
        p_, f_ = np.mgrid[0:128, 0:512]
        cmask[t] = (f_ >= p_ + 128 * t).astype(np.float32)
    bvb = np.ascontiguousarray(
        np.broadcast_to(f("bv")[None, :], (128, D)).copy())
    onesrow = np.ones((1, 128), np.float32)
    return {
        "cmask": cmask, "bvb": bvb, "onesrow": onesrow,
        "wqr": wqr, "wkr": wkr, "wvr": wvr, "wor": wor,
        "w1r": w1r, "w3r": w3r, "w2r": w2r,
        "bq": f("bq"), "bk": f("bk"), "bv": f("bv"), "bo": f("bo"),
        "b1": f("b1"), "b3": f("b3"), "b2": f("b2"),
        "g1": f("ln1_g"), "be1": f("ln1_b"), "g2": f("ln2_g"), "be2": f("ln2_b"),
        "cost": cos, "sint": sin, "rl": rl, "ident": ident,
    }


def kernel(**inputs):
    from concourse.bass_utils import run_bass_kernel_spmd

    if "nc" not in _CACHE:
        _CACHE["nc"] = _build()
    nc = _CACHE["nc"]

    shared = _host_inputs(inputs)
    x = np.asarray(inputs["x"], np.float32)
    in_maps = [dict(shared, x=np.ascontiguousarray(x[b])) for b in range(B)]
    res = run_bass_kernel_spmd(nc, in_maps, list(range(B))).results
    out = np.stack([res[b]["out"] for b in range(B)], axis=0)
    return out.astype(np.float32)
